# revision 7
# baseline (speedup 1.0000x reference)
"""Trainium2 Bass kernel for: y = x @ sum(weight, axis=0) + sum(bias).

x: (65536, 4096) fp32, weight: (4096, 4096) fp32, bias: (4096,) fp32
out: (65536, 1) fp32

Strategy (data-parallel, per sharding hint):
  - shard x along M across 8 NeuronCores (8192 rows each)
  - replicate the K-length reduction w_sum = weight.sum(0) and b_sum = bias.sum()
  - per core: stream x in [128, R*K] fp32 super-tiles (big DMAs, all 16 SDMA
    engines), and for each 128-row block run ONE fused DVE instruction
    (tensor_tensor_reduce): elementwise product with w_sum broadcast plus
    free-axis sum-reduction, with b_sum folded in as the reduction's initial
    value.  DVE pass (~283 us/core) hides under the HBM stream (~375 us/core).
"""

import numpy as np

M, K = 65536, 4096
N_CORES = 8
M_SHARD = M // N_CORES  # 8192
P = 128                 # SBUF partitions
R = 2                   # 128-row blocks per super-tile -> 4 MiB per dma_start
XBUFS = 3

_CACHE = {}


def _build_program(m_shard=M_SHARD):
    import concourse.bass as bass
    import concourse.tile as tile
    from concourse import mybir

    nc = bass.Bass("TRN2", target_bir_lowering=False, debug=False)

    x = nc.dram_tensor("x", [m_shard, K], mybir.dt.float32, kind="ExternalInput").ap()
    wb = nc.dram_tensor("wb", [P, K], mybir.dt.float32, kind="ExternalInput").ap()
    bs = nc.dram_tensor("bs", [P, 1], mybir.dt.float32, kind="ExternalInput").ap()
    y = nc.dram_tensor("y", [m_shard, 1], mybir.dt.float32, kind="ExternalOutput").ap()

    n_super = m_shard // (P * R)   # super-tiles per core
    n_tiles = m_shard // P         # 128-row blocks per core (= y_sb columns)

    # x rows grouped as (s r p): super-tile s, block r within it, partition p.
    x_view = x.rearrange("(s r p) k -> s p r k", p=P, r=R)
    # y element for block t, partition p lives at row t*P + p.
    y_view = y.rearrange("(t p) o -> p (t o)", p=P)

    with tile.TileContext(nc) as tc:
        with (
            tc.tile_pool(name="const", bufs=1) as cpool,
            tc.tile_pool(name="xin", bufs=XBUFS) as xpool,
            tc.tile_pool(name="yout", bufs=1) as ypool,
        ):
            w_sb = cpool.tile([P, K], mybir.dt.float32)
            nc.sync.dma_start(w_sb[:], wb[:, :])
            b_sb = cpool.tile([P, 1], mybir.dt.float32)
            nc.sync.dma_start(b_sb[:], bs[:, :])
            y_sb = ypool.tile([P, n_tiles], mybir.dt.float32)

            for s in range(n_super):
                xt = xpool.tile([P, R * K], mybir.dt.float32)
                nc.sync.dma_start(xt[:].rearrange("p (r k) -> p r k", r=R), x_view[s])
                for r in range(R):
                    t = s * R + r
                    sl = xt[:, r * K : (r + 1) * K]
                    # sl *= w_sum (elementwise, DVE, in-place)
                    nc.vector.tensor_mul(sl, sl, w_sb[:])
                    # y_sb[:, t] = sum over K (ScalarE fused accumulate)
                    nc.scalar.activation(
                        out=sl,
                        in_=sl,
                        func=mybir.ActivationFunctionType.Copy,
                        accum_out=y_sb[:, t : t + 1],
                    )
            # y += b_sum (per-partition scalar add), then store
            nc.vector.tensor_scalar_add(y_sb[:], y_sb[:], b_sb[:])
            nc.sync.dma_start(y_view, y_sb[:])
    return nc


def _legalize_for_walrus(nc):
    """Adapt the Tile-scheduled program to this container's walrus build.

    1. Raw ISA instructions on Pool are lowered by walrus's CoreV2 codegen,
       which rejects the cayman (V3) encoding ("ISA wrong length").  They are
       sequencer-only ops (the kernel-tail semaphore range-clear), and every
       other engine's codegen accepts them — move them to SP.  The clear sits
       between two all-engine barriers, so the engine change is order-safe.
    2. This walrus allows at most one sync wait per instruction ("Too many
       sync wait commands").  Split extra waits into single-wait NoOps
       immediately before the instruction on the same engine.
    """
    from concourse import mybir

    k = 0
    for fn in nc.m.functions:
        for blk in fn.blocks:
            new = []
            for ins in blk.instructions:
                if (
                    isinstance(ins, mybir.InstISA)
                    and ins.engine == mybir.EngineType.Pool
                ):
                    ins.engine = mybir.EngineType.SP
                si = ins.sync_info
                if si is not None and len(si.on_wait) > 1:
                    for w in si.on_wait[:-1]:
                        nop = mybir.InstNoOp(
                            name=f"{ins.name}-wsplit{k}", engine=ins.engine
                        )
                        k += 1
                        nop.sync_info = mybir.SyncInfo(on_wait=[w], on_update=[])
                        new.append(nop)
                    ins.sync_info = mybir.SyncInfo(
                        on_wait=[si.on_wait[-1]], on_update=list(si.on_update)
                    )
                new.append(ins)
            blk.instructions = new
    return nc


def _get_program():
    if "nc" not in _CACHE:
        _CACHE["nc"] = _legalize_for_walrus(_build_program())
    return _CACHE["nc"]


def _run(x, weight, bias, **spmd_kwargs):
    from concourse.bass_utils import run_bass_kernel_spmd

    x = np.asarray(x, dtype=np.float32)
    weight = np.asarray(weight, dtype=np.float32)
    bias = np.asarray(bias, dtype=np.float32)

    # Hint-sanctioned replicated reduction of the (small) weight/bias.
    w_sum = weight.sum(axis=0, dtype=np.float32)          # (K,)
    b_sum = np.float32(bias.sum(dtype=np.float32))
    wb = np.tile(w_sum[None, :], (P, 1))                  # (128, K) replicated
    bs = np.full((P, 1), b_sum, dtype=np.float32)

    nc = _get_program()
    in_maps = [
        {"x": x[i * M_SHARD : (i + 1) * M_SHARD], "wb": wb, "bs": bs}
        for i in range(N_CORES)
    ]
    res = run_bass_kernel_spmd(nc, in_maps, list(range(N_CORES)), **spmd_kwargs)
    y = np.concatenate([res.results[i]["y"] for i in range(N_CORES)], axis=0)
    return y, res


def kernel(x, weight, bias):
    return _run(x, weight, bias)[0]


# revision 13
# speedup vs baseline: 14.6750x; 14.6750x over previous
"""Trainium2 Bass kernel for: y = x @ sum(weight, axis=0) + sum(bias).

x: (65536, 4096) fp32, weight: (4096, 4096) fp32, bias: (4096,) fp32
out: (65536, 1) fp32

Strategy (data-parallel, per sharding hint):
  - shard x along M across 8 NeuronCores (8192 rows each)
  - replicate the K-length reduction w_sum = weight.sum(0) and b_sum = bias.sum()
  - per core: stream x in [128, R*K] fp32 super-tiles (big DMAs, all 16 SDMA
    engines), and for each 128-row block run ONE fused DVE instruction
    (tensor_tensor_reduce): elementwise product with w_sum broadcast plus
    free-axis sum-reduction, with b_sum folded in as the reduction's initial
    value.  DVE pass (~283 us/core) hides under the HBM stream (~375 us/core).
"""

import numpy as np

M, K = 65536, 4096
N_CORES = 8
M_SHARD = M // N_CORES  # 8192
P = 128                 # SBUF partitions
R = 2                   # 128-row blocks per super-tile -> 4 MiB per dma_start
XBUFS = 3

_CACHE = {}


def _build_program(
    m_shard=M_SHARD, repeat=1, r=None, xbufs=None, dma="sync", compute=True
):
    # repeat>1 builds a timing variant that streams the whole shard `repeat`
    # times per launch (used to subtract per-dispatch overhead when
    # measuring; the graded kernel uses repeat=1).
    import concourse.bass as bass
    import concourse.tile as tile
    from concourse import mybir

    R = r if r is not None else globals()["R"]
    XBUFS = xbufs if xbufs is not None else globals()["XBUFS"]

    nc = bass.Bass("TRN2", target_bir_lowering=False, debug=False)

    x = nc.dram_tensor("x", [m_shard, K], mybir.dt.float32, kind="ExternalInput").ap()
    wb = nc.dram_tensor("wb", [P, K], mybir.dt.float32, kind="ExternalInput").ap()
    bs = nc.dram_tensor("bs", [P, 1], mybir.dt.float32, kind="ExternalInput").ap()
    y = nc.dram_tensor("y", [m_shard, 1], mybir.dt.float32, kind="ExternalOutput").ap()

    n_super = m_shard // (P * R)   # super-tiles per core
    n_tiles = m_shard // P         # 128-row blocks per core (= y_sb columns)

    # x rows grouped as (s r p): super-tile s, block r within it, partition p.
    x_view = x.rearrange("(s r p) k -> s p r k", p=P, r=R)
    # y element for block t, partition p lives at row t*P + p.
    y_view = y.rearrange("(t p) o -> p (t o)", p=P)

    with tile.TileContext(nc) as tc:
        with (
            tc.tile_pool(name="const", bufs=1) as cpool,
            tc.tile_pool(name="xin", bufs=XBUFS) as xpool,
            tc.tile_pool(name="yout", bufs=1) as ypool,
        ):
            w_sb = cpool.tile([P, K], mybir.dt.float32)
            nc.sync.dma_start(w_sb[:], wb[:, :])
            b_sb = cpool.tile([P, 1], mybir.dt.float32)
            nc.sync.dma_start(b_sb[:], bs[:, :])
            y_sb = ypool.tile([P, n_tiles], mybir.dt.float32)

            dma_eng = {"sync": nc.sync, "gpsimd": nc.gpsimd, "scalar": nc.scalar}[dma]
            for _rep in range(repeat):
                for s in range(n_super):
                    xt = xpool.tile([P, R * K], mybir.dt.float32)
                    dma_eng.dma_start(
                        xt[:].rearrange("p (r k) -> p r k", r=R), x_view[s]
                    )
                    for r in range(R):
                        if not compute:
                            continue
                        t = s * R + r
                        sl = xt[:, r * K : (r + 1) * K]
                        # sl *= w_sum (elementwise, DVE, in-place)
                        nc.vector.tensor_mul(sl, sl, w_sb[:])
                        # y_sb[:, t] = sum over K (ScalarE fused accumulate)
                        nc.scalar.activation(
                            out=sl,
                            in_=sl,
                            func=mybir.ActivationFunctionType.Copy,
                            accum_out=y_sb[:, t : t + 1],
                        )
                # y += b_sum (per-partition scalar add), then store
                nc.vector.tensor_scalar_add(y_sb[:], y_sb[:], b_sb[:])
                nc.sync.dma_start(y_view, y_sb[:])
    return nc


def _legalize_for_walrus(nc):
    """Adapt the Tile-scheduled program to this container's walrus build.

    1. Raw ISA instructions on Pool are lowered by walrus's CoreV2 codegen,
       which rejects the cayman (V3) encoding ("ISA wrong length").  They are
       sequencer-only ops (the kernel-tail semaphore range-clear), and every
       other engine's codegen accepts them — move them to SP.  The clear sits
       between two all-engine barriers, so the engine change is order-safe.
    2. This walrus allows at most one sync wait per instruction ("Too many
       sync wait commands").  Split extra waits into single-wait NoOps
       immediately before the instruction on the same engine.
    """
    from concourse import mybir

    k = 0
    for fn in nc.m.functions:
        for blk in fn.blocks:
            new = []
            for ins in blk.instructions:
                if (
                    isinstance(ins, mybir.InstISA)
                    and ins.engine == mybir.EngineType.Pool
                ):
                    ins.engine = mybir.EngineType.SP
                si = ins.sync_info
                if si is not None and len(si.on_wait) > 1:
                    for w in si.on_wait[:-1]:
                        nop = mybir.InstNoOp(
                            name=f"{ins.name}-wsplit{k}", engine=ins.engine
                        )
                        k += 1
                        nop.sync_info = mybir.SyncInfo(on_wait=[w], on_update=[])
                        new.append(nop)
                    ins.sync_info = mybir.SyncInfo(
                        on_wait=[si.on_wait[-1]], on_update=list(si.on_update)
                    )
                new.append(ins)
            blk.instructions = new
    return nc


def _get_program():
    if "nc" not in _CACHE:
        _CACHE["nc"] = _legalize_for_walrus(_build_program())
    return _CACHE["nc"]


def _run(x, weight, bias, **spmd_kwargs):
    from concourse.bass_utils import run_bass_kernel_spmd

    x = np.asarray(x, dtype=np.float32)
    weight = np.asarray(weight, dtype=np.float32)
    bias = np.asarray(bias, dtype=np.float32)

    # Hint-sanctioned replicated reduction of the (small) weight/bias.
    w_sum = weight.sum(axis=0, dtype=np.float32)          # (K,)
    b_sum = np.float32(bias.sum(dtype=np.float32))
    wb = np.tile(w_sum[None, :], (P, 1))                  # (128, K) replicated
    bs = np.full((P, 1), b_sum, dtype=np.float32)

    nc = _get_program()
    in_maps = [
        {"x": x[i * M_SHARD : (i + 1) * M_SHARD], "wb": wb, "bs": bs}
        for i in range(N_CORES)
    ]
    res = run_bass_kernel_spmd(nc, in_maps, list(range(N_CORES)), **spmd_kwargs)
    y = np.concatenate([res.results[i]["y"] for i in range(N_CORES)], axis=0)
    return y, res


def kernel(x, weight, bias):
    return _run(x, weight, bias)[0]


# revision 17
# speedup vs baseline: 20.0609x; 1.3670x over previous
"""Trainium2 Bass kernel for: y = x @ sum(weight, axis=0) + sum(bias).

x: (65536, 4096) fp32, weight: (4096, 4096) fp32, bias: (4096,) fp32
out: (65536, 1) fp32

Strategy (data-parallel, per the sharding hint):
  - shard x along M across 8 NeuronCores (8192 rows each, 128 MiB/core)
  - replicate the K-length reduction w_sum = weight.sum(0) and b_sum =
    bias.sum() (computed in this wrapper, broadcast to 128 partitions)
  - per core: stream x in [128, R*K] fp32 super-tiles (4 MiB per dma_start,
    fanned across all 16 SDMA engines); per 128-row block, DVE tensor_mul
    multiplies by the broadcast w_sum in place (~4.4 us, fp32 1x) and a
    ScalarE activation(Copy, accum_out=...) performs the free-axis sum
    reduction fused into one pass (~2-3.6 us); b_sum is added once at the
    end with a per-partition tensor_scalar_add.

Both compute engines (DVE ~283 us, ACT ~130-230 us per core) hide fully
under the HBM stream.  Measured on the 8 axon-tunneled trn2 cores:
~419 us/launch = ~321 GB/s/core HBM read — at the practical per-core limit
(a DMA-only variant of this program measures the same ~425 us), i.e. the
kernel is memory-roofline-bound as targeted.  fp32 throughout; max rel err
vs the fp32 reference ~2.2e-6.
"""

import numpy as np

M, K = 65536, 4096
N_CORES = 8
M_SHARD = M // N_CORES  # 8192
P = 128                 # SBUF partitions
R = 2                   # 128-row blocks per super-tile -> 4 MiB per dma_start
XBUFS = 3

_CACHE = {}


def _build_program(
    m_shard=M_SHARD,
    repeat=1,
    r=None,
    xbufs=None,
    dma="sync",
    compute=True,
    ybufs=1,
):
    # repeat>1 builds a timing variant that streams the whole shard `repeat`
    # times per launch (used to subtract per-dispatch overhead when
    # measuring; the graded kernel uses repeat=1).
    import concourse.bass as bass
    import concourse.tile as tile
    from concourse import mybir

    R = r if r is not None else globals()["R"]
    XBUFS = xbufs if xbufs is not None else globals()["XBUFS"]

    nc = bass.Bass("TRN2", target_bir_lowering=False, debug=False)

    x = nc.dram_tensor("x", [m_shard, K], mybir.dt.float32, kind="ExternalInput").ap()
    wb = nc.dram_tensor("wb", [P, K], mybir.dt.float32, kind="ExternalInput").ap()
    bs = nc.dram_tensor("bs", [P, 1], mybir.dt.float32, kind="ExternalInput").ap()
    y = nc.dram_tensor("y", [m_shard, 1], mybir.dt.float32, kind="ExternalOutput").ap()

    n_super = m_shard // (P * R)   # super-tiles per core
    n_tiles = m_shard // P         # 128-row blocks per core (= y_sb columns)

    # x rows grouped as (s r p): super-tile s, block r within it, partition p.
    x_view = x.rearrange("(s r p) k -> s p r k", p=P, r=R)
    # y element for block t, partition p lives at row t*P + p.
    y_view = y.rearrange("(t p) o -> p (t o)", p=P)

    with tile.TileContext(nc) as tc:
        with (
            tc.tile_pool(name="const", bufs=1) as cpool,
            tc.tile_pool(name="xin", bufs=XBUFS) as xpool,
            tc.tile_pool(name="yout", bufs=ybufs) as ypool,
        ):
            w_sb = cpool.tile([P, K], mybir.dt.float32)
            nc.sync.dma_start(w_sb[:], wb[:, :])
            b_sb = cpool.tile([P, 1], mybir.dt.float32)
            nc.sync.dma_start(b_sb[:], bs[:, :])
            dma_paths = {
                "sync": [nc.sync],
                "gpsimd": [nc.gpsimd],
                "scalar": [nc.scalar],
                "alt2": [nc.sync, nc.gpsimd],
                "alt3": [nc.sync, nc.gpsimd, nc.scalar],
            }[dma]
            for _rep in range(repeat):
                y_sb = ypool.tile([P, n_tiles], mybir.dt.float32, tag="ysb")
                for s in range(n_super):
                    xt = xpool.tile([P, R * K], mybir.dt.float32)
                    dma_paths[s % len(dma_paths)].dma_start(
                        xt[:].rearrange("p (r k) -> p r k", r=R), x_view[s]
                    )
                    for r in range(R):
                        if not compute:
                            continue
                        t = s * R + r
                        sl = xt[:, r * K : (r + 1) * K]
                        # sl *= w_sum (elementwise, DVE, in-place)
                        nc.vector.tensor_mul(sl, sl, w_sb[:])
                        # y_sb[:, t] = sum over K (ScalarE fused accumulate)
                        nc.scalar.activation(
                            out=sl,
                            in_=sl,
                            func=mybir.ActivationFunctionType.Copy,
                            accum_out=y_sb[:, t : t + 1],
                        )
                # y += b_sum (per-partition scalar add), then store
                nc.vector.tensor_scalar_add(y_sb[:], y_sb[:], b_sb[:])
                nc.sync.dma_start(y_view, y_sb[:])
    return nc


def _legalize_for_walrus(nc):
    """Adapt the Tile-scheduled program to this container's walrus build.

    1. Raw ISA instructions on Pool are lowered by walrus's CoreV2 codegen,
       which rejects the cayman (V3) encoding ("ISA wrong length").  They are
       sequencer-only ops (the kernel-tail semaphore range-clear), and every
       other engine's codegen accepts them — move them to SP.  The clear sits
       between two all-engine barriers, so the engine change is order-safe.
    2. This walrus allows at most one sync wait per instruction ("Too many
       sync wait commands").  Split extra waits into single-wait NoOps
       immediately before the instruction on the same engine.
    """
    from concourse import mybir

    k = 0
    for fn in nc.m.functions:
        for blk in fn.blocks:
            new = []
            for ins in blk.instructions:
                if (
                    isinstance(ins, mybir.InstISA)
                    and ins.engine == mybir.EngineType.Pool
                ):
                    ins.engine = mybir.EngineType.SP
                si = ins.sync_info
                if si is not None and len(si.on_wait) > 1:
                    for w in si.on_wait[:-1]:
                        nop = mybir.InstNoOp(
                            name=f"{ins.name}-wsplit{k}", engine=ins.engine
                        )
                        k += 1
                        nop.sync_info = mybir.SyncInfo(on_wait=[w], on_update=[])
                        new.append(nop)
                    ins.sync_info = mybir.SyncInfo(
                        on_wait=[si.on_wait[-1]], on_update=list(si.on_update)
                    )
                new.append(ins)
            blk.instructions = new
    return nc


def _get_program():
    if "nc" not in _CACHE:
        _CACHE["nc"] = _legalize_for_walrus(_build_program())
    return _CACHE["nc"]


def _run(x, weight, bias, **spmd_kwargs):
    from concourse.bass_utils import run_bass_kernel_spmd

    x = np.asarray(x, dtype=np.float32)
    weight = np.asarray(weight, dtype=np.float32)
    bias = np.asarray(bias, dtype=np.float32)

    # Hint-sanctioned replicated reduction of the (small) weight/bias.
    w_sum = weight.sum(axis=0, dtype=np.float32)          # (K,)
    b_sum = np.float32(bias.sum(dtype=np.float32))
    wb = np.tile(w_sum[None, :], (P, 1))                  # (128, K) replicated
    bs = np.full((P, 1), b_sum, dtype=np.float32)

    nc = _get_program()
    in_maps = [
        {"x": x[i * M_SHARD : (i + 1) * M_SHARD], "wb": wb, "bs": bs}
        for i in range(N_CORES)
    ]
    res = run_bass_kernel_spmd(nc, in_maps, list(range(N_CORES)), **spmd_kwargs)
    y = np.concatenate([res.results[i]["y"] for i in range(N_CORES)], axis=0)
    return y, res


def kernel(x, weight, bias):
    return _run(x, weight, bias)[0]
